# revision 1
# baseline (speedup 1.0000x reference)
"""Self-contained TRN2 Bass kernel for nn_FLoRALayer (B=8, S=2048, D=1024, R=8).

kernel(**inputs) takes FULL unsharded inputs:
    x         [8, 2048, 1024] f32
    adapter_b [8, 1024, 8]    f32
    adapter_a [8, 8, 1024]    f32
    W0        [1024, 1024]    f32
returns the FULL [8, 2048, 1024] f32 output of:
    BxW0 = einsum('bsd,bdr,do->bsro', x.astype(fp16), adapter_b, W0)
    out  = relu(mean(swapaxes(adapter_a,1,2)[:,None]*BxW0.reshape(b,s,d,r), -1))

Math refactor (verified exactly): with o = kk*128 + g*16 + mp,
    W_eff[dd, o] = adapter_b[dd, kk] * sum_rp adapter_a[rp, o] * W0[dd, (o%128)*8 + rp]
    out[b] = relu((x_fp16[b] @ W_eff[b]) / 8)
which is one [2048,1024] @ [1024,1024] matmul per batch -- data-parallel over
the batch dim: batch b runs on NeuronCore b (sharding_hint's layout).

Host does ONLY data placement (sharding/permutation/replication, no
arithmetic): X^T and W0^T tile-packing, a block-diagonal embedding of
adapter_a (A_sp), and 16x replication of adapter_b (B_bc).

v2 schedule (from the v1 trace: first matmul at t=15.5us, 5.6us tail drain):
  - ALL input DMAs ride the SP (sync) HWDGE queue in priority order
    w0_0, asp, x0, bbc, w0_1, x1, w0_2, x2, w0_3, x3, w0_4..w0_7, x4..x15
    so W0 chunks stream in at a steady cadence and x never blocks them.
  - Stores ride Act (h0 halves) and Pool/SWDGE (h1 halves) exclusively --
    input loads can never queue behind a store trigger's semaphore wait.
  - DVE queue order gives W0-chunk casts + BTT (W_eff build) priority over
    x casts; per chunk: [BTT(t), w0t_cast(t+1)] pipelines the chain
    dma -> cast -> C-matmul -> BTT at ~1.5us/chunk.
  - PSUM split: pmm_w (6 banks, 3 s-tiles in flight) + pmm_c (2 banks for
    the C chain) so pool ring reuse cannot cross-block the two chains.
  - 3 warm s-tiles interleave with the W0 chain (consume chunk t-1 right
    after BTT(t-1)), the other 13 run back-to-back afterward.
"""

from contextlib import ExitStack

import numpy as np

S, D, R = 2048, 1024, 8
NT = D // 128
NS = S // 128
WARM = [0, 1, 2]
N_CORES = 8

_compiled = None


def _build_kernel():
    import concourse.bass as bass
    import concourse.tile as tile
    from concourse import bacc, mybir

    F32 = mybir.dt.float32
    F16 = mybir.dt.float16

    nc = bacc.Bacc(
        "TRN2", target_bir_lowering=False, debug=False, num_devices=N_CORES
    )

    x_d = nc.dram_tensor("xtp", [NS, 128, D], F32, kind="ExternalInput").ap()
    w0_d = nc.dram_tensor("w0tp", [NT, 128, D], F32, kind="ExternalInput").ap()
    asp_d = nc.dram_tensor("asp", [128, NT * 128], F32, kind="ExternalInput").ap()
    bbc_d = nc.dram_tensor("bbc", [128, NT * 128], F32, kind="ExternalInput").ap()
    out_d = nc.dram_tensor("out", [S, D], F32, kind="ExternalOutput").ap()

    with tile.TileContext(nc) as tc, ExitStack() as ctx:
        pool = lambda name, bufs, **kw: ctx.enter_context(
            tc.tile_pool(name=name, bufs=bufs, **kw)
        )
        const_p = pool("const", 1)
        w0stage_p = pool("w0stage", 8)
        w0t_p = pool("w0t", 1)
        weff_p = pool("weff", 1)
        xstage_p = pool("xstage", 8)
        xth_p = pool("xth", 5)
        outst_p = pool("outst", 5)
        pmm_w = pool("pmmw", 6, space="PSUM")
        pmm_c = pool("pmmc", 2, space="PSUM")

        import concourse.mybir as mybir_mod

        # ---------- SP queue: every input DMA, in priority order ----------
        w0s_tiles = []

        def w0_dma(t):
            w0s = w0stage_p.tile([128, D], F32, tag="w0s", name=f"w0s{t}")
            nc.sync.dma_start(w0s[:], w0_d[t])
            w0s_tiles.append(w0s)

        xs_tiles = {}

        def x_dma(s):
            xs = xstage_p.tile([128, D], F32, tag="xs", name=f"xs{s}")
            nc.sync.dma_start(xs[:], x_d[s])
            xs_tiles[s] = xs

        asp_st = const_p.tile([128, NT * 128], F32, tag="asp_st")
        bbc = const_p.tile([128, NT * 128], F32, tag="bbc")

        # split the critical-path w0_0 / asp transfers into halves so the
        # first C-build matmuls start while the DMA queue is still ramping
        # (~160 GB/s for the first ~5us); bbc's first 128 cols ride a tiny
        # DMA so BTT(0) isn't gated on the full bbc transfer
        w0s0 = w0stage_p.tile([128, D], F32, tag="w0s", name="w0s0")
        nc.sync.dma_start(w0s0[:, 0:512], w0_d[0][:, 0:512])
        nc.sync.dma_start(asp_st[:, 0:512], asp_d[:, 0:512])
        nc.sync.dma_start(bbc[:, 0:128], bbc_d[:, 0:128])
        nc.sync.dma_start(w0s0[:, 512:1024], w0_d[0][:, 512:1024])
        nc.sync.dma_start(asp_st[:, 512:1024], asp_d[:, 512:1024])
        w0s_tiles.append(w0s0)
        w0_dma(1)
        nc.sync.dma_start(bbc[:, 128 : NT * 128], bbc_d[:, 128 : NT * 128])
        w0_dma(2)
        x_dma(0)
        w0_dma(3)
        x_dma(1)
        w0_dma(4)
        x_dma(2)
        w0_dma(5)
        x_dma(3)
        w0_dma(6)
        w0_dma(7)
        for s in range(4, NS):
            x_dma(s)

        # ---------- SBUF fp16 targets ----------
        w0t = w0t_p.tile([128, NT * 1024], F16, tag="w0t")
        weff = weff_p.tile([128, NT * 1024], F16, tag="weff")
        asp_h = const_p.tile([128, NT * 128], F16, tag="asp_h")

        xth_tiles = {}

        def x_cast(s):
            xth = xth_p.tile([128, D], F16, tag="xth", name=f"xth{s}")
            nc.vector.tensor_copy(xth[:], xs_tiles[s][:])
            xth_tiles[s] = xth

        def w0_cast(t):
            # DVE: Pool-engine casts measured 3.7us each (5.4x DVE) -- keep
            # the cast chain on DVE, ordered behind each BTT
            nc.vector.tensor_copy(
                w0t[:, t * 1024 : (t + 1) * 1024], w0s_tiles[t][:]
            )

        # ---------- per-chunk builders ----------
        pcs_tiles = {}

        def c_alloc(t):
            pcs_tiles[t] = [
                pmm_c.tile([128, 512], F32, tag="pmmc", name=f"pc{t}_{i}")
                for i in range(2)
            ]

        F32R = mybir.dt.float32r

        def c_mms(t, gs=range(NT), f32r=False):
            # f32r: contract the raw f32 staged tiles directly (skips the
            # DVE casts -- used for the head chunks where PE is idle and
            # the DVE queue is the critical path); precision ~bf16-grade,
            # far inside the 2e-2 gate
            if t not in pcs_tiles:
                c_alloc(t)
            pcs = pcs_tiles[t]
            for g in gs:
                if f32r:
                    lhsT = w0s_tiles[t][:, g * 128 : (g + 1) * 128].bitcast(F32R)
                    rhs = asp_st[:, g * 128 : (g + 1) * 128].bitcast(F32R)
                else:
                    lhsT = w0t[:, t * 1024 + g * 128 : t * 1024 + (g + 1) * 128]
                    rhs = asp_h[:, g * 128 : (g + 1) * 128]
                nc.tensor.matmul(
                    pcs[g // 4][:, (g % 4) * 128 : (g % 4 + 1) * 128],
                    lhsT=lhsT,
                    rhs=rhs,
                    start=True,
                    stop=True,
                )

        def btt(t):
            pcs = pcs_tiles[t]
            wv = weff[:, t * 1024 : (t + 1) * 1024].rearrange(
                "p (kk g mp) -> p kk g mp", kk=8, g=NT, mp=16
            )
            for half in range(2):
                wvh = wv[:, :, half * 4 : (half + 1) * 4, :]
                bv = bbc[:, t * 128 : (t + 1) * 128].rearrange(
                    "p (kk mp) -> p kk mp", kk=8
                )[:, :, None, :].broadcast_to([128, 8, 4, 16])
                pv = pcs[half].rearrange("p (g kk mp) -> p kk g mp", g=4, kk=8)
                nc.vector.tensor_tensor(
                    out=wvh, in0=pv, in1=bv, op=mybir_mod.AluOpType.mult
                )

        def mm_pair(po, xth, c):
            for h in range(2):
                nc.tensor.matmul(
                    po[h][:],
                    lhsT=xth[:, c * 128 : (c + 1) * 128],
                    rhs=weff[:, c * 1024 + h * 512 : c * 1024 + (h + 1) * 512],
                    start=(c == 0),
                    stop=(c == NT - 1),
                )

        def evac_store(s, po):
            # full-tile store: 4KB-contiguous DRAM rows run at full queue
            # BW; the half-tile (2KB row / 4KB stride) pattern caps at
            # ~131 GB/s per queue.  Alternate Act / Pool trigger queues.
            outst = outst_p.tile([128, D], F32, tag="outst", name=f"outst{s}")
            nc.scalar.activation(
                outst[:, 0:512],
                po[0][:],
                mybir_mod.ActivationFunctionType.Relu,
                scale=0.125,
            )
            nc.vector.tensor_scalar(
                out=outst[:, 512:1024],
                in0=po[1][:],
                scalar1=0.125,
                scalar2=0.0,
                op0=mybir_mod.AluOpType.mult,
                op1=mybir_mod.AluOpType.max,
            )
            # last tile rides the HWDGE (Act) queue: SWDGE's final DRAIN
            # costs ~3.2us extra
            eng = nc.gpsimd if s % 2 == 1 and s != NS - 1 else nc.scalar
            eng.dma_start(out_d[s * 128 : (s + 1) * 128, :], outst[:])

        # ---------- warm phase: W_eff build + 3 warm s-tiles ----------
        po_warm = {
            s: [
                pmm_w.tile([128, 512], F32, tag="pmmw", name=f"po{s}_{i}")
                for i in range(2)
            ]
            for s in WARM
        }

        # chunk 0 at half granularity: each half's cast + C matmuls gate
        # only on that half's DMA (f32r would skip the casts, but the BIR
        # verifier requires f32r inputs to be pre-rounded by a producer op)
        nc.vector.tensor_copy(w0t[:, 0:512], w0s_tiles[0][:, 0:512])
        nc.vector.tensor_copy(asp_h[:, 0:512], asp_st[:, 0:512])
        c_mms(0, range(4))
        nc.vector.tensor_copy(w0t[:, 512:1024], w0s_tiles[0][:, 512:1024])
        nc.vector.tensor_copy(asp_h[:, 512:1024], asp_st[:, 512:1024])
        c_mms(0, range(4, NT))
        btt(0)
        w0_cast(1)
        x_cast(0)

        # warm tiles consume chunks as they are built; s=2 joins at t=2
        # (its x tile lands later) and catches up at 2 chunks/iteration.
        # DVE order per chunk: cast(t+1) ahead of btt(t) hides the PE
        # C-matmul latency inside the DVE queue.
        cursor = {s: 0 for s in WARM}
        for t in range(1, NT):
            # C(t) first on the PE queue: it fills the BTT(t-1) window
            # (its only deps are cast(t) and the pcs ring, both satisfied);
            # the warm pairs that need BTT(t-1) follow it.
            c_mms(t)
            # DVE order flips per regime: early chunks are DMA-bound, so a
            # cast-ahead would block BTT(t) behind the next W0 arrival;
            # late chunks are chain-bound, so the cast must run during C(t)
            if t <= 3:
                btt(t)
                if t + 1 < NT:
                    w0_cast(t + 1)
            else:
                if t + 1 < NT:
                    w0_cast(t + 1)
                btt(t)
            if t == 1:
                x_cast(1)
            if t == 2:
                x_cast(2)
            for s in WARM:
                if s == 2 and t < 3:
                    continue
                emitted = 0
                while cursor[s] < t and emitted < 2:
                    mm_pair(po_warm[s], xth_tiles[s], cursor[s])
                    cursor[s] += 1
                    emitted += 1
        for s in WARM:
            while cursor[s] < NT:
                mm_pair(po_warm[s], xth_tiles[s], cursor[s])
                cursor[s] += 1

        for s in WARM:
            evac_store(s, po_warm[s])

        # ---------- main phase: remaining 13 s-tiles ----------
        # cycle pmm_c back in (free after BTT(7)) so 4 tiles stay in flight
        for s in range(len(WARM), NS):
            for tgt in (s, s + 1, s + 2):
                if tgt < NS and tgt not in xth_tiles:
                    x_cast(tgt)
            mpool = pmm_c if (s - len(WARM)) % 4 == 3 else pmm_w
            tg = "pmmc" if mpool is pmm_c else "pmmw"
            po = [
                mpool.tile([128, 512], F32, tag=tg, name=f"po{s}_{i}")
                for i in range(2)
            ]
            for c in range(NT):
                mm_pair(po, xth_tiles[s], c)
            evac_store(s, po)

    nc.compile()
    return nc


def _pack_inputs(x_b, adapter_b_b, adapter_a_b, W0):
    """Pure data placement (permutation / replication / zero-padding)."""
    xtp = np.ascontiguousarray(
        x_b.reshape(NS, 128, NT, 128).transpose(0, 3, 2, 1).reshape(NS, 128, D),
        np.float32,
    )
    w0tp = np.ascontiguousarray(
        W0.reshape(NT, 128, NT, 128).transpose(0, 3, 2, 1).reshape(NT, 128, D),
        np.float32,
    )
    asp = np.zeros((NT, 128, 128), np.float32)
    aa = adapter_a_b
    for g in range(NT):
        for mp in range(16):
            for rp in range(R):
                asp[g, mp * 8 + rp, np.arange(8) * 16 + mp] = aa[
                    rp, np.arange(8) * 128 + g * 16 + mp
                ]
    asp = np.ascontiguousarray(asp.transpose(1, 0, 2).reshape(128, NT * 128))
    bbc = np.repeat(adapter_b_b, 16, axis=1).reshape(D, 128)
    bbc = np.ascontiguousarray(
        bbc.reshape(NT, 128, 128).transpose(1, 0, 2).reshape(128, NT * 128),
        np.float32,
    )
    return {"xtp": xtp, "w0tp": w0tp, "asp": asp, "bbc": bbc}


def kernel(x, adapter_b, adapter_a, W0):
    global _compiled
    x = np.asarray(x, np.float32)
    adapter_b = np.asarray(adapter_b, np.float32)
    adapter_a = np.asarray(adapter_a, np.float32)
    W0 = np.asarray(W0, np.float32)
    B = x.shape[0]
    assert B == N_CORES and x.shape == (B, S, D)

    if _compiled is None:
        _compiled = _build_kernel()

    from concourse.bass_utils import run_bass_kernel_spmd

    in_maps = [
        _pack_inputs(x[b], adapter_b[b], adapter_a[b], W0) for b in range(B)
    ]
    res = run_bass_kernel_spmd(_compiled, in_maps, list(range(N_CORES)))
    out = np.stack([res.results[b]["out"] for b in range(B)]).astype(np.float32)
    return out

